# revision 37
# baseline (speedup 1.0000x reference)
"""Causal self-attention (B=1, T=4096, C=1024, H=16) on 8 trn2 NeuronCores.

Sharding: tensor-parallel over heads — 2 heads per core. Each core computes
q/k/v for its 2 heads from the full sequence, runs causal flash-style
attention fully on-chip, and produces a partial output projection
(its heads' contribution y_h @ W_proj[head_rows]); the host sums the 8
bf16 partials in f32 (the unshard step); b_proj is added on the host.

v2 schedule (vs v1): the per-(head,ipair) chain QK->exp->PV is software-
pipelined so the in-order PE queue never waits on a fresh exp: PV runs one
ipair behind QK, QK h0/h1 are emitted adjacently (row-packed concurrent
via tile_position), and PE slack inside the ACT-bound attention stream is
filled with next-super QKV pieces and prev-super projection pieces. The
softmax denominators are reciprocated on 128 lanes (DRAM-transposed
[128,8] layout) instead of a 3.3us single-lane [1,512] DVE reciprocal.

Per-core layouts:
  qT, kT  [dhead(2 heads stacked)=128, T] bf16
  v       [T, .] bf16, per-l-tile slots [v_h0|1|pad|v_h1|1|pad]; the
          constant-1 columns make the P@V matmul also emit the softmax
          denominators (row 64 of each head's [65,512] psum output).
  S^T     [l, q] per (head, l-tile pair, q-super); exp'd by ACT into bf16
          P^T with no max-subtraction (|logits| small; fp32 exp only
          overflows past ~88). Two l-tiles share one [128,1024] psum tile.
  oT      [d=128, q] — normalized by 1/l then projected (K=128 matmul).
"""

import numpy as np
from contextlib import ExitStack

import concourse.bass as bass
import concourse.mybir as mybir
import concourse.tile as tile
from concourse.bass import AP
from concourse.masks import make_identity

T = 4096
C = 1024
H = 16
HD = 64
NCORES = 8
SUP = 512           # q super-block width
NSUP = T // SUP
LTN = T // 128      # number of 128-row l-tiles
VSLOT = 130         # v slot: [v_h0(0:64)|1(64)|v_h1(65:129)|1(129)]

F32 = mybir.dt.float32
BF16 = mybir.dt.bfloat16
AF = mybir.ActivationFunctionType
ALU = mybir.AluOpType


def _split_multi_waits(nc, max_waits=1):
    """The walrus build here rejects >1 semaphore wait on one CTRL
    instruction; push excess waits onto preceding same-engine NoOps."""
    n_new = 0
    for f in nc.m.functions:
        for bb in f.blocks:
            out = []
            changed = False
            for ins in bb.instructions:
                si = ins.sync_info
                waits = list(si.on_wait) if si is not None else []
                if len(waits) > max_waits:
                    changed = True
                    excess, keep = waits[:-max_waits], waits[-max_waits:]
                    for ci in range(0, len(excess), max_waits):
                        n_new += 1
                        out.append(mybir.InstNoOp(
                            name=f"{ins.name}-ws{n_new}",
                            engine=ins.engine, ins=[], outs=[],
                            sync_info=mybir.SyncInfo(
                                on_wait=excess[ci:ci + max_waits], on_update=[]),
                        ))
                    ins.sync_info = mybir.SyncInfo(
                        on_wait=keep, on_update=list(si.on_update))
                out.append(ins)
            if changed:
                bb.instructions = out
    return n_new


def build_nc(split_waits=True):
    nc = bass.Bass("TRN2")
    xT = nc.dram_tensor("xT", [C, T], BF16, kind="ExternalInput")
    wq = nc.dram_tensor("wq", [C, 128], BF16, kind="ExternalInput")
    wk = nc.dram_tensor("wk", [C, 128], BF16, kind="ExternalInput")
    wv = nc.dram_tensor("wv", [C, 128], BF16, kind="ExternalInput")
    bq = nc.dram_tensor("bq", [128, 1], F32, kind="ExternalInput")
    bk = nc.dram_tensor("bk", [128, 1], F32, kind="ExternalInput")
    bv = nc.dram_tensor("bv", [128, 1], F32, kind="ExternalInput")
    wp = nc.dram_tensor("wp", [128, C], BF16, kind="ExternalInput")
    out_d = nc.dram_tensor("out", [T, C], BF16, kind="ExternalOutput")

    with tile.TileContext(nc) as tc:
        with ExitStack() as ctx:
            P = lambda **kw: ctx.enter_context(tc.tile_pool(**kw))
            const_p = P(name="const", bufs=1)
            qk_p = P(name="qk", bufs=1)
            v_p = P(name="v", bufs=1)
            x_p = P(name="x", bufs=3)
            vt_p = P(name="vt", bufs=2)
            pt_p = P(name="pt", bufs=8)
            ot_sb_p = P(name="ot_sb", bufs=2)
            ep_p = P(name="ep", bufs=4)
            rl_p = P(name="rl", bufs=2)
            dram_p = P(name="dram", bufs=2, space="DRAM")

            # ---- constants ----
            # x super 0 goes out on the vector engine's DMA queue so it
            # overlaps the weight DMAs (the sync engine issues serially);
            # biases/wp ride on scalar/gpsimd queues for the same reason.
            x0_sb = x_p.tile([128, 8, SUP], BF16, tag="x", name="x0")
            nc.scalar.dma_start(
                x0_sb[:],
                xT[:, 0:SUP].rearrange("(ck p) t -> p ck t", p=128))
            wq_sb = const_p.tile([128, 8, 128], BF16)
            wk_sb = const_p.tile([128, 8, 128], BF16)
            wv_sb = const_p.tile([128, 8, 128], BF16)
            for w_sb, w_d in ((wq_sb, wq), (wk_sb, wk), (wv_sb, wv)):
                nc.sync.dma_start(
                    w_sb[:], w_d[:].rearrange("(ck p) m -> p ck m", p=128))
            bq_sb = const_p.tile([128, 1], F32)
            bk_sb = const_p.tile([128, 1], F32)
            bv_sb = const_p.tile([128, 1], F32)
            for b_sb, b_d in ((bq_sb, bq), (bk_sb, bk), (bv_sb, bv)):
                nc.gpsimd.dma_start(b_sb[:], b_d[:])
            wp_sb = const_p.tile([128, C], BF16)
            nc.gpsimd.dma_start(wp_sb[:], wp[:])

            qT = qk_p.tile([128, T], BF16)
            kT = qk_p.tile([128, T], BF16)
            v_sb = v_p.tile([128, LTN * VSLOT], BF16)
            # only the per-slot ones-columns (64, 129) need initializing;
            # the pack copies fill the v parts
            v_view = v_sb[:].rearrange("p (l c) -> p l c", c=VSLOT)
            nc.gpsimd.memset(v_view[:, :, 64], 1.0)
            nc.gpsimd.memset(v_view[:, :, 129], 1.0)
            ident = const_p.tile([128, 128], BF16)
            make_identity(nc, ident[:])
            ones_row = const_p.tile([1, 128], BF16)
            nc.gpsimd.memset(ones_row[:], 1.0)

            qkv_ps = P(name="qkv_ps", bufs=1, space="PSUM")
            st_ps = P(name="st_ps", bufs=2, space="PSUM")
            ot_ps_p = P(name="ot_ps", bufs=2, space="PSUM")
            pj_ps = P(name="pj_ps", bufs=1, space="PSUM")

            # per-super state carried between emission phases
            ot_ps = {}      # j -> [ot_h0, ot_h1] psum tiles
            ot_f = {}       # j -> [65, 2, SUP] f32 sbuf evac of ot psum
            ot_sb = {}      # j -> normalized [128, SUP] bf16 sbuf tile
            rc_row = {}     # j -> [1, 2, SUP] bf16 1/l rows

            # ---------------- emission pieces ----------------
            def piece_xdma(s):
                x_sb = x_p.tile([128, 8, SUP], BF16, tag="x", name=f"x{s}")
                nc.sync.dma_start(
                    x_sb[:],
                    xT[:, s * SUP:(s + 1) * SUP].rearrange(
                        "(ck p) t -> p ck t", p=128))
                return x_sb

            def piece_qkv(s, x_sb, which):
                w_sb, b_sb = {"q": (wq_sb, bq_sb), "k": (wk_sb, bk_sb),
                              "v": (wv_sb, bv_sb)}[which]
                ps = qkv_ps.tile([128, SUP], F32, tag="qkv",
                                 name=f"qkv{s}{which}")
                for ck in range(8):
                    nc.tensor.matmul(
                        ps[:], lhsT=w_sb[:, ck, :], rhs=x_sb[:, ck, :],
                        start=(ck == 0), stop=(ck == 7))
                if which == "q":
                    # (q + bias) * 1/sqrt(hd) folded here
                    nc.vector.tensor_scalar(
                        out=qT[:, s * SUP:(s + 1) * SUP], in0=ps[:],
                        scalar1=bq_sb[:], scalar2=1.0 / np.sqrt(HD),
                        op0=ALU.add, op1=ALU.mult)
                    return None
                elif which == "k":
                    nc.vector.tensor_scalar_add(
                        out=kT[:, s * SUP:(s + 1) * SUP], in0=ps[:],
                        scalar1=bk_sb[:])
                    return None
                else:
                    vt_sb = vt_p.tile([128, SUP], BF16, tag="vt",
                                      name=f"vt{s}")
                    nc.vector.tensor_scalar_add(
                        out=vt_sb[:], in0=ps[:], scalar1=bv_sb[:])
                    return vt_sb

            def piece_vtp(s, vt_sb, lt_loc):
                # transpose one 128-col block of v and pack both head
                # halves into the v slot layout with a single 3D copy
                lt = s * (SUP // 128) + lt_loc
                blk = slice(lt_loc * 128, (lt_loc + 1) * 128)
                tp = qkv_ps.tile([128, 128], BF16, tag="qkv",
                                 name=f"tp{s}_{lt_loc}")
                nc.tensor.transpose(tp[:], vt_sb[:, blk], ident[:])
                src = tp[:].rearrange("p (h d) -> p h d", h=2)
                dst = v_sb[:, lt * VSLOT:(lt + 1) * VSLOT].rearrange(
                    "p (h d) -> p h d", d=65)[:, :, 0:64]
                nc.vector.tensor_copy(dst, src)

            def emit_qk(j, i, st_tiles):
                # one l-tile, both heads side by side in one [128,2,512]
                # psum tile (2 banks). Both matmuls share the tile's WAR
                # wait (long satisfied), so the row-disjoint pair streams
                # concurrently in the PE (row packing).
                n0 = max(0, 128 * (i - 4 * j))
                s_ps = st_ps.tile([128, 2, SUP], F32, tag="st",
                                  name=f"st{j}_{i}")
                st_tiles[0] = s_ps
                for h in range(2):
                    hs = slice(h * 64, (h + 1) * 64)
                    nc.tensor.matmul(
                        s_ps[:, h, n0:SUP],
                        lhsT=kT[hs, i * 128:(i + 1) * 128],
                        rhs=qT[hs, j * SUP + n0:(j + 1) * SUP],
                        start=True, stop=True,
                        tile_position=(h * 64, 0))

            def emit_exp(j, i, st_tiles, pt_tiles):
                # single [128,1024] exp covers both heads; stale psum cols
                # left of the causal trim get exp'd harmlessly (never read)
                pt = pt_p.tile([128, 2, SUP], BF16, tag="pt",
                               name=f"pt{j}_{i}")
                pt_tiles[0] = pt
                nc.scalar.activation(pt[:, :, :], st_tiles[0][:, :, :],
                                     AF.Exp)
                if i >= 4 * j:  # diagonal: mask strictly-upper block
                    n0 = 128 * (i - 4 * j)
                    for h in range(2):
                        nc.gpsimd.affine_select(
                            out=pt[:, h, n0:n0 + 128],
                            in_=pt[:, h, n0:n0 + 128],
                            compare_op=ALU.is_ge, fill=0.0, base=0,
                            channel_multiplier=-1, pattern=[[1, 128]])

            def emit_pv(j, i, pt_tiles, nlt):
                n0 = max(0, 128 * (i - 4 * j))
                for h in range(2):
                    nc.tensor.matmul(
                        ot_ps[j][h][0:65, n0:SUP],
                        lhsT=v_sb[:, i * VSLOT + h * 65:
                                  i * VSLOT + (h + 1) * 65],
                        rhs=pt_tiles[0][:, h, n0:SUP],
                        start=(i == 0), stop=(i == nlt - 1))

            def emit_denom_act(j):
                # evac ot psum -> sbuf (releases the psum banks fast), then
                # 1/l = exp(-ln(l)) on ACT rows (ln+exp share one table
                # set) — no DMA hops, everything stays on-chip.
                of = rl_p.tile([65, 2, SUP], F32, tag="otf", name=f"otf{j}")
                ot_f[j] = of
                nc.vector.tensor_copy(of[:, 0, :], ot_ps[j][0][0:65, :])
                nc.vector.tensor_copy(of[:, 1, :], ot_ps[j][1][0:65, :])
                lnl = rl_p.tile([1, 2, SUP], F32, tag="ln", name=f"ln{j}")
                nc.scalar.activation(lnl[:], of[64:65, :, :], AF.Ln)
                rc = rl_p.tile([1, 2, SUP], BF16, tag="rc", name=f"rc{j}")
                nc.scalar.activation(rc[:], lnl[:], AF.Exp, scale=-1.0)
                rc_row[j] = rc

            def piece_normh(j, h):
                # partition-broadcast 1/l via a K=1 ones-matmul into the
                # pj bank, then scale that head's 64 output rows
                pj = pj_ps.tile([128, 512], F32, tag="pj",
                                name=f"bc{j}_{h}")
                nc.tensor.matmul(pj[:], lhsT=ones_row[:],
                                 rhs=rc_row[j][:, h, :],
                                 start=True, stop=True)
                if h == 0:
                    ot_sb[j] = ot_sb_p.tile([128, SUP], BF16, tag="osb",
                                            name=f"osb{j}")
                nc.vector.tensor_tensor(
                    out=ot_sb[j][h * 64:(h + 1) * 64, :],
                    in0=ot_f[j][0:64, h, :],
                    in1=pj[0:64, :], op=ALU.mult)

            def piece_proj(j, tb, half, pool=None):
                pj = (pool or pj_ps).tile(
                    [128, 512], F32,
                    tag=("qkv" if pool is qkv_ps else "pj"),
                    name=f"pj{j}_{tb}_{half}")
                nc.tensor.matmul(
                    pj[:],
                    lhsT=ot_sb[j][:, tb * 128:(tb + 1) * 128],
                    rhs=wp_sb[:, half * 512:(half + 1) * 512],
                    start=True, stop=True)
                res = ep_p.tile([128, 512], BF16, tag="res",
                                name=f"res{j}_{tb}_{half}")
                nc.vector.tensor_copy(res[:], pj[:])
                nc.sync.dma_start(
                    out_d[j * SUP + tb * 128:j * SUP + (tb + 1) * 128,
                          half * 512:(half + 1) * 512],
                    res[:])

            def tail_state(j):
                ot_f[j] = rl_p.tile([65, 2, SUP], F32, tag="otf",
                                    name=f"otf{j}")
                rc_row[j] = rl_p.tile([1, 2, SUP], BF16, tag="rc",
                                      name=f"rc{j}")
                tail_state.lnl = rl_p.tile([1, 2, SUP], F32, tag="ln",
                                           name=f"ln{j}")
                ot_sb[j] = ot_sb_p.tile([128, SUP], BF16, tag="osb",
                                        name=f"osb{j}")

            def tail_half(j, c0, c1, tbs):
                # epilogue for q columns [c0,c1) of the last super; the
                # first half runs while the final l-tiles still stream
                of, rc, lnl = ot_f[j], rc_row[j], tail_state.lnl
                nc.vector.tensor_copy(of[:, 0, c0:c1],
                                      ot_ps[j][0][0:65, c0:c1])
                nc.vector.tensor_copy(of[:, 1, c0:c1],
                                      ot_ps[j][1][0:65, c0:c1])
                nc.scalar.activation(lnl[:, :, c0:c1], of[64:65, :, c0:c1],
                                     AF.Ln)
                nc.scalar.activation(rc[:, :, c0:c1], lnl[:, :, c0:c1],
                                     AF.Exp, scale=-1.0)
                n = c1 - c0
                for h in range(2):
                    pool = qkv_ps if h else pj_ps
                    pj = pool.tile([128, 512], F32,
                                   tag=("qkv" if pool is qkv_ps else "pj"),
                                   name=f"tbc{j}_{c0}_{h}")
                    nc.tensor.matmul(pj[:, 0:n], lhsT=ones_row[:],
                                     rhs=rc[:, h, c0:c1],
                                     start=True, stop=True)
                    nc.vector.tensor_tensor(
                        out=ot_sb[j][h * 64:(h + 1) * 64, c0:c1],
                        in0=of[0:64, h, c0:c1],
                        in1=pj[0:64, 0:n], op=ALU.mult)
                for tb in tbs:
                    for half in range(2):
                        piece_proj(j, tb, half,
                                   pool=(qkv_ps if (tb * 2 + half) % 2
                                         else pj_ps))

            # ---------------- prologue: A_0 ----------------
            # super 0's x arrives as 8 per-chunk DMAs so the first matmul
            # can start after ~1 chunk instead of the full 1MB transfer;
            # v first so the v-transpose/pack chain (PV(0)'s dep) overlaps
            # the q/k pieces.
            x_cur = x0_sb
            piece_qkv(0, x_cur, "q")
            piece_qkv(0, x_cur, "k")
            vt0 = piece_qkv(0, x_cur, "v")
            for lt_loc in range(4):
                piece_vtp(0, vt0, lt_loc)

            # ---------------- main loop ----------------
            for j in range(NSUP):
                nlt = 4 * j + 4
                ot_ps[j] = [ot_ps_p.tile([128, SUP], F32, tag="ot",
                                         name=f"ot{j}_{hh}")
                            for hh in range(2)]

                # filler pieces to absorb PE slack in the attention stream:
                # "early" = next-super QKV (ready to run), "late" = pieces
                # that depend on the prev super's recip chain (normalize,
                # proj) — placed in the last-half slots so the chain's DMA
                # latency never head-of-line-blocks the PE queue.
                early, late = [], []
                if j + 1 < NSUP:
                    s = j + 1
                    xs = piece_xdma(s)
                    early.append(lambda s=s, xs=xs: piece_qkv(s, xs, "q"))
                    early.append(lambda s=s, xs=xs: piece_qkv(s, xs, "k"))

                    def v_and_tp(s=s, xs=xs):
                        vt = piece_qkv(s, xs, "v")
                        piece_vtp(s, vt, 0)
                        piece_vtp(s, vt, 1)
                        return vt
                    holder = {}
                    early.append(lambda s=s, h=holder: h.__setitem__(
                        "vt", v_and_tp(s)))
                    early.append(lambda s=s, h=holder: (
                        piece_vtp(s, h["vt"], 2), piece_vtp(s, h["vt"], 3)))
                if j - 1 >= 0:
                    late.append(lambda jj=j - 1: piece_normh(jj, 0))
                    late.append(lambda jj=j - 1: piece_normh(jj, 1))
                    for tb in range(4):
                        for half in range(2):
                            late.append(
                                lambda jj=j - 1, tb=tb, hf=half:
                                piece_proj(jj, tb, hf))

                half_i = nlt // 2
                ei = li = 0
                prev_pt = None
                prev_i = None
                for i in range(nlt):
                    st_tiles = [None]
                    pt_tiles = [None]
                    emit_qk(j, i, st_tiles)
                    emit_exp(j, i, st_tiles, pt_tiles)
                    if prev_pt is not None:
                        emit_pv(j, prev_i, prev_pt, nlt)
                    if i < half_i or li >= len(late):
                        slots = max(1, half_i - i)
                        want = (len(early) - ei + slots - 1) // slots
                        for _ in range(max(0, want)):
                            if ei < len(early):
                                early[ei]()
                                ei += 1
                    else:
                        slots = nlt - i
                        want = (len(late) - li + slots - 1) // slots
                        for _ in range(max(0, want)):
                            if li < len(late):
                                late[li]()
                                li += 1
                    if j == NSUP - 1 and prev_i == nlt - 3:
                        tail_state(j)
                        tail_half(j, 0, SUP // 2, (0, 1))
                    prev_pt, prev_i = pt_tiles, i
                emit_pv(j, prev_i, prev_pt, nlt)
                if j == NSUP - 1:
                    tail_half(j, SUP // 2, SUP, (2, 3))
                while ei < len(early):
                    early[ei]()
                    ei += 1
                while li < len(late):
                    late[li]()
                    li += 1
                if j < NSUP - 1:
                    emit_denom_act(j)

    if split_waits:
        _split_multi_waits(nc, 1)
    return nc


_NC_CACHE = {}


def _get_nc():
    if "nc" not in _NC_CACHE:
        _NC_CACHE["nc"] = build_nc()
    return _NC_CACHE["nc"]


def make_in_maps(x, W_attn, b_attn, W_proj, b_proj):
    import ml_dtypes
    bf = ml_dtypes.bfloat16
    x = np.ascontiguousarray(np.asarray(x, dtype=np.float32)).reshape(T, C)
    W_attn = np.asarray(W_attn, dtype=np.float32)
    b_attn = np.asarray(b_attn, dtype=np.float32)
    W_proj = np.asarray(W_proj, dtype=np.float32)
    xT = np.ascontiguousarray(x.T).astype(bf)
    in_maps = []
    for c in range(NCORES):
        sl = slice(128 * c, 128 * (c + 1))
        m = {
            "xT": xT,
            "wq": np.ascontiguousarray(W_attn[:, sl]).astype(bf),
            "wk": np.ascontiguousarray(W_attn[:, C:][:, sl]).astype(bf),
            "wv": np.ascontiguousarray(W_attn[:, 2 * C:][:, sl]).astype(bf),
            "bq": np.ascontiguousarray(b_attn[sl]).reshape(128, 1),
            "bk": np.ascontiguousarray(b_attn[C:][sl]).reshape(128, 1),
            "bv": np.ascontiguousarray(b_attn[2 * C:][sl]).reshape(128, 1),
            "wp": np.ascontiguousarray(W_proj[sl, :]).astype(bf),
        }
        in_maps.append(m)
    return in_maps


def kernel(x, W_attn, b_attn, W_proj, b_proj):
    from concourse.bass_utils import run_bass_kernel_spmd
    nc = _get_nc()
    in_maps = make_in_maps(x, W_attn, b_attn, W_proj, b_proj)
    res = run_bass_kernel_spmd(nc, in_maps, core_ids=list(range(NCORES)))
    acc = np.zeros((T, C), dtype=np.float32)
    for c in range(NCORES):
        acc += np.asarray(res.results[c]["out"], dtype=np.float32)
    acc += np.asarray(b_proj, dtype=np.float32)  # bias folded into unshard
    return acc.reshape(1, T, C)
